# revision 4
# baseline (speedup 1.0000x reference)
"""Longformer self-attention (BART-style) Trainium2 kernel, v2.

Sharding: 8 cores = 2 batches x 4 head-groups (4 heads each).

v2 redesign vs baseline:
  - Single interleaved emission schedule: window-attention QK blocks (one
    head at a time) are emitted between projection PSUM groups so the
    scalar-engine EXPs hide under projection matmuls and the PE array never
    idles (stays at 2.4GHz).
  - Packed band slots: the two half-masked edge key-chunks (d=-2, d=+3)
    share one 256-wide score slot (query halves); global-key scores ride in
    a normal slot; one EXP per 3-slot group.
  - Softmax normalization via in-SBUF reciprocal + ones-matmul partition
    broadcast + fused multiply into ctxT (no DRAM roundtrip / gpsimd DMA).
  - Host pre-layouts weights/x-chunks so every DMA is one contiguous
    descriptor per partition (sprayed across all 16 DMA engines).
  - Output chunk 0 split into cols [64:512] (gathered early) and [0:64]
    (after global attention) so the AllGather tail is tiny; gather-path DMAs
    ride the gpsimd queue so they never block x-chunk loads.
"""
import sys
import numpy as np

sys.path.insert(0, "/opt/trn_rl_repo")

import ml_dtypes

import concourse.bass as bass
import concourse.bacc as bacc
import concourse.tile as tile
from concourse import mybir
from concourse.bass_utils import run_bass_kernel_spmd

BF16 = ml_dtypes.bfloat16
B, S, E, H, D, W, G = 2, 4096, 1024, 16, 64, 256, 64
QB = 256           # query block for window attention
NKC = S // 128     # 32 key chunks
NQB = S // QB      # 16 query blocks
dt = mybir.dt
AF = mybir.ActivationFunctionType

MASK_IDS = {"packed": 0, "packed_qb1": 1, "m1": 2, "m2": 3, "glob0": 4}

VARIANT = "default"
DEBUG_DUMP = False


def qb_plan(qb):
    """Slot plan for query block qb.

    Returns list of groups; each group is a list of slots; each slot is
    (subtasks, mask_id) with subtasks a list of (kc, qlo, qhi); kc == -1
    denotes the global-key slot (keys 0:G with standard projections).
    """
    base = 2 * qb
    slots = []
    sub = []
    if base - 2 >= 0:
        sub.append((base - 2, 0, 128))
    if base + 3 < NKC:
        sub.append((base + 3, 128, 256))
    if sub:
        mask = "packed_qb1" if qb == 1 else "packed"
        slots.append((sub, MASK_IDS[mask]))
    for d, mname in ((-1, "m1"), (0, None), (1, None), (2, "m2")):
        kc = base + d
        if 0 <= kc < NKC:
            m = mname
            if kc == 0 and d == 0:
                m = "glob0"          # qb0: zero rows < G (handled globally)
            slots.append(([(kc, 0, 256)], MASK_IDS[m] if m else None))
    slots.append(([(-1, 0, 256)], None))   # global keys
    return [slots[0:3], slots[3:6]]


def _build_masks():
    j = np.arange(128)[:, None]   # key-in-chunk (partition)
    q = np.arange(256)[None, :]   # query-in-block (free)
    packed = np.where(q < 128, j >= q, j <= q - 128)
    packed_qb1 = np.where(q < 128, (j >= q) & (j >= 64), j <= q - 128)
    m1 = (j >= q - 128) & (q < 999)
    m2 = (j <= q) & (q < 999)
    glob0 = (j >= 64) & (q < 999)
    return np.stack([packed, packed_qb1, m1, m2, glob0]).astype(BF16)


def _build_bass():
    nc = bacc.Bacc("TRN2", num_devices=8)

    def inp(name, shape, dtype=dt.bfloat16):
        return nc.declare_dram_parameter(name, list(shape), dtype, isOutput=False)

    xt = inp("xt", [8, 128, 4096])              # [chunk, p, kt*512] host-packed
    wq = inp("wq", [128, 2048])                 # [p, kt*256], pre-scaled 1/8
    wk = inp("wk", [128, 2048])
    wv = inp("wv", [128, 2048])
    wkg = inp("wkg", [128, 2048])
    wvg = inp("wvg", [128, 2048])
    wqg = inp("wqg", [128, 2048])               # pre-scaled by 1/8
    wo = inp("wo", [128, 2048])                 # E-column slice of Wo
    bq = inp("bq", [128, 2], dt.float32)        # pre-scaled by 1/8
    bk = inp("bk", [128, 2], dt.float32)
    bkg = inp("bkg", [128, 2], dt.float32)
    bqg = inp("bqg", [128, 2], dt.float32)      # pre-scaled by 1/8
    bow = inp("bow", [128, 2], dt.float32)      # bo + bv@Wo   (col slice)
    bog = inp("bog", [128, 2], dt.float32)      # bo + bvg@Wo  (col slice)
    masks = inp("masks", [128, 5, 256])         # bf16 0/1 band masks
    outt = nc.declare_dram_parameter("outt", [2, 128, S], dt.float32, isOutput=True)
    dbg = None
    if DEBUG_DUMP:
        dbg = dict(
            dctx=nc.declare_dram_parameter("dctx", [128, 2, S], dt.bfloat16, isOutput=True),
            dv=nc.declare_dram_parameter("dv", [128, 32, 4, 65], dt.bfloat16, isOutput=True),
            dk=nc.declare_dram_parameter("dk", [128, 2, S], dt.bfloat16, isOutput=True),
            dq=nc.declare_dram_parameter("dq", [128, 2, S], dt.bfloat16, isOutput=True))

    with tile.TileContext(nc) as tc:
        _emit(tc, xt, wq, wk, wv, wkg, wvg, wqg, wo, bq, bk, bkg, bqg,
              bow, bog, masks, outt, dbg)
    nc.compile()
    return nc


class Emitter:
    def __init__(self, tc, xt, weights, biases, masks, outt):
        self.nc = tc.nc
        self.tc = tc
        nc = self.nc
        import contextlib
        self.ctx = contextlib.ExitStack()
        ctx = self.ctx

        self.xt, self.masks_in, self.outt = xt, masks, outt

        persist = ctx.enter_context(tc.tile_pool(name="persist", bufs=1))
        self.persist = persist
        self.xchunk = ctx.enter_context(tc.tile_pool(name="xchunk", bufs=2))
        self.ptiles = ctx.enter_context(tc.tile_pool(name="ptiles", bufs=10))
        self.gtiles = ctx.enter_context(tc.tile_pool(name="gtiles", bufs=2))
        self.rbpool = ctx.enter_context(tc.tile_pool(name="rbpool", bufs=2))
        self.otile = ctx.enter_context(tc.tile_pool(name="otile", bufs=2))
        self.dram = ctx.enter_context(tc.tile_pool(name="dram", bufs=2, space="DRAM"))
        self.psum = ctx.enter_context(tc.tile_pool(name="psum", bufs=2, space="PSUM"))

        # ---- weight / bias / mask loads (contiguous per partition) -------
        self.w_sb = {}
        for name, t in weights.items():
            sb = persist.tile([128, 8, 256], dt.bfloat16, name=name + "_sb")
            nc.sync.dma_start(out=sb, in_=t[:].rearrange("p (kt m) -> p kt m", kt=8))
            self.w_sb[name] = sb
        self.b_sb = {}
        for name, t in biases.items():
            sb = persist.tile([128, 2], dt.float32, name=name + "_sb")
            nc.sync.dma_start(out=sb, in_=t[:])
            self.b_sb[name] = sb
        self.mask_sb = persist.tile([128, 5, 256], dt.bfloat16, name="mask_sb")
        nc.sync.dma_start(out=self.mask_sb, in_=masks[:])

        # ---- persistent activation tiles ---------------------------------
        self.qT = persist.tile([128, 2, S], dt.bfloat16, name="qT")
        self.kT = persist.tile([128, 2, S], dt.bfloat16, name="kT")
        self.kgT = persist.tile([128, 2, S], dt.bfloat16, name="kgT")
        self.qgT = persist.tile([128, 2, G], dt.bfloat16, name="qgT")
        self.v_sb = persist.tile([128, 32, 4, 65], dt.bfloat16, name="v")
        self.vg_sb = persist.tile([128, 32, 4, 65], dt.bfloat16, name="vg")
        self.ctxT = persist.tile([128, 2, S], dt.bfloat16, name="ctxT")
        self.ones = persist.tile([128, 64], dt.bfloat16, name="ones")

        nc.vector.memset(self.v_sb[:, :, :, 64:65], 1.0)
        nc.vector.memset(self.vg_sb[:, :, :, 64:65], 1.0)
        nc.vector.memset(self.ones, 1.0)

        self.xc = [None] * 8

    # ---- DMA of an x chunk (sync queue) ----------------------------------
    def load_xc(self, c):
        xc = self.xchunk.tile([128, 8, 512], dt.bfloat16, tag="xc", name="xc")
        self.nc.sync.dma_start(
            out=xc, in_=self.xt[c].rearrange("p (kt s) -> p kt s", kt=8))
        self.xc[c] = xc

    # ---- one transposed projection group: dst[:, hp, cs] -----------------
    def proj_T(self, c, wname, bname, dst, hp):
        nc = self.nc
        cs = slice(c * 512, c * 512 + 512)
        w_sb, b_sb, xc = self.w_sb[wname], self.b_sb[bname], self.xc[c]
        ps = self.psum.tile([128, 512], dt.float32, tag="proj", name="ps_proj")
        for kt in range(8):
            nc.tensor.matmul(
                ps, w_sb[:, kt, hp * 128:hp * 128 + 128],
                xc[:, kt, :], start=(kt == 0), stop=(kt == 7))
        nc.scalar.activation(dst[:, hp, cs], ps, AF.Identity,
                             bias=b_sb[:, hp:hp + 1])

    # ---- one natural-layout v/vg group: dst[:, s32, :, 0:64] -------------
    def proj_V(self, c, wname, dst, sc):
        nc = self.nc
        w_sb, xc = self.w_sb[wname], self.xc[c]
        s32 = c * 4 + sc
        ps = self.psum.tile([128, 512], dt.float32, tag="proj", name="ps_projv")
        psn = ps[:, 0:256]
        for kt in range(8):
            nc.tensor.matmul(
                psn, xc[:, kt, sc * 128:sc * 128 + 128],
                w_sb[:, kt, :], start=(kt == 0), stop=(kt == 7))
        nc.scalar.activation(
            dst[:, s32, :, 0:64],
            psn.rearrange("p (h d) -> p h d", h=4), AF.Copy)

    def proj_qg(self, hp):
        nc = self.nc
        ps = self.psum.tile([128, 512], dt.float32, tag="proj", name="ps_qg")
        psg = ps[:, 0:G]
        for kt in range(8):
            nc.tensor.matmul(
                psg, self.w_sb["wqg"][:, kt, hp * 128:hp * 128 + 128],
                self.xc[0][:, kt, 0:G], start=(kt == 0), stop=(kt == 7))
        nc.scalar.activation(self.qgT[:, hp, :], psg, AF.Identity,
                             bias=self.b_sb["bqg"][:, hp:hp + 1])

    # ---- window attention: QK+exp+masks for one (block, head) ------------
    def window_qk_hi(self, qb, hi):
        nc = self.nc
        q0 = qb * QB
        hp, row = hi // 2, (hi % 2) * 64
        groups = qb_plan(qb)
        pTs = []
        for g in groups:
            n = len(g)
            pss = self.psum.tile([128, 3, QB], dt.float32, tag="scores",
                                 name="pss")
            for si, (subs, mi) in enumerate(g):
                for (kc, qlo, qhi) in subs:
                    if kc == -1:
                        nc.tensor.matmul(
                            pss[0:G, si, qlo:qhi],
                            self.kT[row:row + 64, hp, 0:G],
                            self.qT[row:row + 64, hp, q0 + qlo:q0 + qhi],
                            start=True, stop=True)
                    else:
                        nc.tensor.matmul(
                            pss[:, si, qlo:qhi],
                            self.kT[row:row + 64, hp,
                                    kc * 128:kc * 128 + 128],
                            self.qT[row:row + 64, hp, q0 + qlo:q0 + qhi],
                            start=True, stop=True)
            pT = self.ptiles.tile([128, 3, QB], dt.bfloat16, tag="pT",
                                  name="pT")
            nc.scalar.activation(pT[:, 0:n, :], pss[:, 0:n, :], AF.Exp)
            for si, (subs, mi) in enumerate(g):
                if mi is not None:
                    nc.vector.tensor_mul(pT[:, si, :], pT[:, si, :],
                                         self.mask_sb[:, mi, :])
            pTs.append(pT)
        return (qb, q0, hi, hp, row, groups, pTs)

    # ---- window attention: PV + normalization for all 4 heads ------------
    def window_pv(self, states):
        nc = self.nc
        qb, q0 = states[0][0], states[0][1]
        groups = states[0][5]
        tasks = []
        for gi, g in enumerate(groups):
            for si, (subs, mi) in enumerate(g):
                for (kc, qlo, qhi) in subs:
                    tasks.append((kc, qlo, qhi, gi, si))
        # full-range slots first so PSUM has_written is fully seeded
        tasks.sort(key=lambda t: (t[2] - t[1] != QB))
        lo = G if qb == 0 else 0

        fins = []

        def one_pv(st):
            _, _, hi, hp, row, _, pTs = st
            pv = self.psum.tile([128, QB], dt.float32, tag="pv", name="pv")
            for j, (kc, qlo, qhi, gi, si) in enumerate(tasks):
                pT = pTs[gi]
                if kc == -1:
                    lhs = self.v_sb[0:G, 0, hi, :]
                    rhs = pT[0:G, si, qlo:qhi]
                else:
                    lhs = self.v_sb[:, kc, hi, :]
                    rhs = pT[:, si, qlo:qhi]
                nc.tensor.matmul(pv[0:65, qlo:qhi], lhs, rhs,
                                 start=(j == 0), stop=(j == len(tasks) - 1))
            fins.append((pv, self.ctxT[row:row + 64, hp, q0 + lo:q0 + QB]))

        # lag-1 software pipeline: PV(h0), PV(h1), fin(h0), PV(h2), fin(h1)...
        one_pv(states[0])
        for i in range(1, 4):
            one_pv(states[i])
            self.normalize(*fins[i - 1], lo, QB)
        self.normalize(*fins[3], lo, QB)

    def normalize(self, pv, ctx_out, lo, width):
        """ctx_out = pv[0:64, lo:width] / pv[64, lo:width] (per query).

        The reciprocal of the denominator row is broadcast across the 64
        feature partitions with a 1-row f32r matmul; the unnormalized ctx
        rows are staged to SBUF on the scalar engine (DVE tensor_tensor
        cannot take two PSUM operands), then multiplied on the vector
        engine."""
        nc = self.nc
        tmp = self.rbpool.tile([128, QB], dt.bfloat16, tag="tmp", name="tmp")
        nc.scalar.activation(tmp[0:64, lo:width], pv[0:64, lo:width], AF.Copy)
        # 1/den on the scalar engine: rec = exp(-ln(den))
        lnd = self.rbpool.tile([128, QB], dt.float32, tag="lnd", name="lnd")
        nc.scalar.activation(lnd[64:65, 0:width], pv[64:65, 0:width], AF.Ln)
        rec16 = self.rbpool.tile([128, QB], dt.bfloat16, tag="rec16",
                                 name="rec16")
        nc.scalar.activation(rec16[64:65, 0:width], lnd[64:65, 0:width],
                             AF.Exp, scale=-1.0)
        psB = self.psum.tile([128, 3, QB], dt.float32, tag="scores",
                             name="psB")
        nc.tensor.matmul(
            psB[0:64, 0, 0:width], self.ones[64:65, :],
            rec16[64:65, 0:width], start=True, stop=True)
        nc.vector.tensor_mul(ctx_out, tmp[0:64, lo:width],
                             psB[0:64, 0, lo:width])

    # ---- global-query attention ------------------------------------------
    def glob_qk_t(self, hi, pgT, t):
        """One third of the global-query QK for head hi."""
        nc = self.nc
        hp, row = hi // 2, (hi % 2) * 64
        kcs = list(range(t * 12, min(32, t * 12 + 12)))
        nslot = (len(kcs) + 3) // 4
        pss = self.psum.tile([128, 3, QB], dt.float32, tag="scores",
                             name="pss_g")
        for i, kc in enumerate(kcs):
            si, qo = i // 4, (i % 4) * G
            nc.tensor.matmul(
                pss[:, si, qo:qo + G],
                self.kgT[row:row + 64, hp, kc * 128:kc * 128 + 128],
                self.qgT[row:row + 64, hp, :], start=True, stop=True)
        nc.scalar.activation(
            pgT[:, t * 12:t * 12 + len(kcs), :].rearrange(
                "p k g -> p (k g)").rearrange("p (s q) -> p s q", q=QB),
            pss[:, 0:nslot, :], AF.Exp)

    def glob_pv(self, hi, pgT):
        nc = self.nc
        hp, row = hi // 2, (hi % 2) * 64
        pv = self.psum.tile([128, QB], dt.float32, tag="pv", name="pvg")
        for kc in range(NKC):
            nc.tensor.matmul(pv[0:65, 0:G], self.vg_sb[:, kc, hi, :],
                             pgT[:, kc, :], start=(kc == 0),
                             stop=(kc == NKC - 1))
        self.normalize(pv, self.ctxT[row:row + 64, hp, 0:G], 0, G)

    # ---- gather (+ gctx load) and output projection ----------------------
    def gather(self, cols, tag):
        """AllGather ctxT[:, :, cols] across the 4-core group (gpsimd queue).
        Also issues the gctx load (in-queue after the collective)."""
        nc = self.nc
        width = cols.stop - cols.start
        cc_in = self.dram.tile([256, width], dt.bfloat16, tag="cc_in" + tag,
                               name="cc_in")
        nc.gpsimd.dma_start(
            out=cc_in[:].rearrange("(hp p) s -> p hp s", p=128),
            in_=self.ctxT[:, :, cols])
        cc_out = self.dram.tile([1024, width], dt.bfloat16, tag="cc_out" + tag,
                                name="cc_out")
        nc.gpsimd.collective_compute(
            "AllGather", mybir.AluOpType.bypass,
            replica_groups=[[0, 1, 2, 3], [4, 5, 6, 7]],
            ins=[cc_in[:].opt()], outs=[cc_out[:].opt()])
        gctx = self.gtiles.tile([128, 8, 512], dt.bfloat16, tag="gctx",
                                name="gctx")
        gctx = gctx[:, :, 0:width]
        nc.gpsimd.dma_start(
            out=gctx, in_=cc_out[:].rearrange("(kt p) s -> p kt s", p=128))
        return gctx

    def out_proj(self, gctx, cols, bname):
        nc = self.nc
        width = cols.stop - cols.start
        for mt in range(2):
            pso = self.psum.tile([128, 512], dt.float32, tag="proj",
                                 name="pso")
            pso = pso[:, 0:width]
            for kt in range(8):
                nc.tensor.matmul(
                    pso, self.w_sb["wo"][:, kt, mt * 128:mt * 128 + 128],
                    gctx[:, kt, :], start=(kt == 0), stop=(kt == 7))
            ot = self.otile.tile([128, 512], dt.float32, tag="ot", name="ot")
            ot = ot[:, 0:width]
            nc.scalar.activation(ot, pso, AF.Identity,
                                 bias=self.b_sb[bname][:, mt:mt + 1])
            nc.gpsimd.dma_start(out=self.outt[mt, :, cols], in_=ot)


def _emit(tc, xt, wq, wk, wv, wkg, wvg, wqg, wo, bq, bk, bkg, bqg,
          bow, bog, masks, outt, dbg=None):
    em = Emitter(
        tc, xt,
        dict(wq=wq, wk=wk, wv=wv, wkg=wkg, wvg=wvg, wqg=wqg, wo=wo),
        dict(bq=bq, bk=bk, bkg=bkg, bqg=bqg, bow=bow, bog=bog),
        masks, outt)

    def proj_groups(c):
        """The 14 tensor-queue work units of projection chunk c."""
        gs = []
        for wn, bn, dst in (("wq", "bq", em.qT), ("wk", "bk", em.kT),
                            ("wkg", "bkg", em.kgT)):
            for hp in range(2):
                gs.append(lambda wn=wn, bn=bn, dst=dst, hp=hp:
                          em.proj_T(c, wn, bn, dst, hp))
        for wn, dst in (("wv", em.v_sb), ("wvg", em.vg_sb)):
            for sc in range(4):
                gs.append(lambda wn=wn, dst=dst, sc=sc:
                          em.proj_V(c, wn, dst, sc))
        return gs

    # ---- prologue: chunks 0 and 1, plus qg ------------------------------
    em.load_xc(0)
    em.load_xc(1)
    for g in proj_groups(0):
        g()
    em.proj_qg(0)
    em.proj_qg(1)
    em.load_xc(2)
    for g in proj_groups(1):
        g()

    # ---- steady loop: chunks 2..7 with interleaved windows/gathers ------
    # pending output projections: (gctx, cols, bname)
    pending = []
    chunk_cols = [slice(64, 512)] + [slice(s * 512, s * 512 + 512)
                                     for s in range(1, 8)]
    for c in range(2, 8):
        if c + 1 < 8:
            em.load_xc(c + 1)
        gs = proj_groups(c)
        qa, qbb = 2 * c - 4, 2 * c - 3

        sts = []
        for hi in range(4):              # qb a: QK per head between proj groups
            sts.append(em.window_qk_hi(qa, hi))
            gs.pop(0)()
        em.window_pv(sts)

        sts = []
        for hi in range(4):              # qb b
            sts.append(em.window_qk_hi(qbb, hi))
            gs.pop(0)()
        if pending:
            em.out_proj(*pending.pop(0))
        em.window_pv(sts)

        for g in gs:                     # remaining 6 projection groups
            g()

        s = c - 2                        # output chunk completed this iter
        gctx = em.gather(chunk_cols[s], "a")
        pending.append((gctx, chunk_cols[s], "bow"))

    # ---- post-loop: qb 12..15, global attention, remaining gathers ------
    pgTs = {}
    for i, qb in enumerate((12, 13, 14, 15)):
        pgTs[i] = em.gtiles.tile([128, 32, G], dt.bfloat16, tag="pgT",
                                 name="pgT", bufs=4)
        sts = []
        for hi in range(4):
            sts.append(em.window_qk_hi(qb, hi))
            if hi < 3:
                em.glob_qk_t(i, pgTs[i], hi)
        if pending:
            em.out_proj(*pending.pop(0))
        em.window_pv(sts)
        if qb == 13:
            gctx = em.gather(chunk_cols[6], "a")
            pending.append((gctx, chunk_cols[6], "bow"))

    gctx7 = em.gather(chunk_cols[7], "a")
    for hi in range(4):
        em.glob_pv(hi, pgTs[hi])
    gctx0b = em.gather(slice(0, 64), "b")
    while pending:
        em.out_proj(*pending.pop(0))
    em.out_proj(gctx7, chunk_cols[7], "bow")
    em.out_proj(gctx0b, slice(0, 64), "bog")

    if dbg is not None:
        em.nc.sync.dma_start(out=dbg["dctx"][:], in_=em.ctxT[:])
        em.nc.sync.dma_start(out=dbg["dv"][:], in_=em.v_sb[:])
        em.nc.sync.dma_start(out=dbg["dk"][:], in_=em.kT[:])
        em.nc.sync.dma_start(out=dbg["dq"][:], in_=em.qT[:])

    em.ctx.close()


def _host_inputs(x, Wq, bq, Wk, bk, Wv, Wqg, bqg, Wkg, bkg, Wvg, Wo, bo_w, bo_g):
    """Build the 8 per-core input maps (all pre-packed for contiguous DMA)."""
    masks = np.ascontiguousarray(
        _build_masks().transpose(1, 0, 2))          # [128, 5, 256]

    def b16(a):
        return np.ascontiguousarray(a, dtype=np.float32).astype(BF16)

    def wpack(Wslice):                               # [1024, 256] -> [128, 2048]
        return np.ascontiguousarray(
            b16(Wslice).reshape(8, 128, 256).transpose(1, 0, 2).reshape(128, 2048))

    def bpack(b):                                    # [256] -> [128, 2]
        return np.ascontiguousarray(b.reshape(2, 128).T.astype(np.float32))

    def xpack(xb):                                   # [4096, 1024] -> [8,128,4096]
        xT = b16(xb.T)                               # [1024, 4096]
        return np.ascontiguousarray(
            xT.reshape(8, 128, 8, 512).transpose(2, 1, 0, 3).reshape(8, 128, 4096))

    xP = [xpack(x[b]) for b in range(B)]
    in_maps = []
    for c in range(8):
        b, hg = c // 4, c % 4
        cs = slice(256 * hg, 256 * hg + 256)
        in_maps.append({
            "xt": xP[b],
            "wq": wpack(Wq[:, cs] * 0.125), "wk": wpack(Wk[:, cs]),
            "wv": wpack(Wv[:, cs]), "wkg": wpack(Wkg[:, cs]),
            "wvg": wpack(Wvg[:, cs]), "wqg": wpack(Wqg[:, cs] * 0.125),
            "wo": wpack(Wo[:, cs]),
            "bq": bpack(bq[cs] * 0.125),
            "bk": bpack(bk[cs]),
            "bkg": bpack(bkg[cs]),
            "bqg": bpack(bqg[cs] * 0.125),
            "bow": bpack(bo_w[cs]),
            "bog": bpack(bo_g[cs]),
            "masks": masks.astype(BF16),
        })
    return in_maps


_CACHE = {}


def kernel(hidden_states, key_value_states, Wq, bq, Wk, bk, Wv, bv,
           Wqg, bqg, Wkg, bkg, Wvg, bvg, Wo, bo, num_heads, window,
           num_global, _trace=False):
    x = np.asarray(hidden_states, np.float32)
    args = [np.asarray(a, np.float32) for a in
            (Wq, bq, Wk, bk, Wv, bv, Wqg, bqg, Wkg, bkg, Wvg, bvg, Wo, bo)]
    Wq, bq, Wk, bk, Wv, bv, Wqg, bqg, Wkg, bkg, Wvg, bvg, Wo, bo = args
    bo_w = bo + bv @ Wo
    bo_g = bo + bvg @ Wo

    if "nc" not in _CACHE:
        _CACHE["nc"] = _build_bass()
    nc = _CACHE["nc"]

    in_maps = _host_inputs(x, Wq, bq, Wk, bk, Wv, Wqg, bqg, Wkg, bkg,
                           Wvg, Wo, bo_w, bo_g)
    res = run_bass_kernel_spmd(nc, in_maps, core_ids=list(range(8)),
                               trace=_trace)
    _CACHE["last_result"] = res

    out = np.zeros((B, S, E), np.float32)
    for c in range(8):
        b, hg = c // 4, c % 4
        ot = np.asarray(res.results[c]["outt"], np.float32)  # [2, 128, S]
        out[b, :, 256 * hg:256 * hg + 256] = ot.reshape(256, S).T
    return out


# revision 5
# speedup vs baseline: 1.1680x; 1.1680x over previous
"""Longformer self-attention (BART-style) Trainium2 kernel, v2.

Sharding: 8 cores = 2 batches x 4 head-groups (4 heads each).

v2 redesign vs baseline:
  - Single interleaved emission schedule: window-attention QK blocks (one
    head at a time) are emitted between projection PSUM groups so the
    scalar-engine EXPs hide under projection matmuls and the PE array never
    idles (stays at 2.4GHz).
  - Packed band slots: the two half-masked edge key-chunks (d=-2, d=+3)
    share one 256-wide score slot (query halves); global-key scores ride in
    a normal slot; one EXP per 3-slot group.
  - Softmax normalization via in-SBUF reciprocal + ones-matmul partition
    broadcast + fused multiply into ctxT (no DRAM roundtrip / gpsimd DMA).
  - Host pre-layouts weights/x-chunks so every DMA is one contiguous
    descriptor per partition (sprayed across all 16 DMA engines).
  - Output chunk 0 split into cols [64:512] (gathered early) and [0:64]
    (after global attention) so the AllGather tail is tiny; gather-path DMAs
    ride the gpsimd queue so they never block x-chunk loads.
"""
import sys
import numpy as np

sys.path.insert(0, "/opt/trn_rl_repo")

import ml_dtypes

import concourse.bass as bass
import concourse.bacc as bacc
import concourse.tile as tile
from concourse import mybir
from concourse.bass_utils import run_bass_kernel_spmd

BF16 = ml_dtypes.bfloat16
B, S, E, H, D, W, G = 2, 4096, 1024, 16, 64, 256, 64
QB = 256           # query block for window attention
NKC = S // 128     # 32 key chunks
NQB = S // QB      # 16 query blocks
dt = mybir.dt
AF = mybir.ActivationFunctionType

MASK_IDS = {"packed": 0, "packed_qb1": 1, "m1": 2, "m2": 3, "glob0": 4}

VARIANT = "default"
DEBUG_DUMP = False


def qb_plan(qb):
    """Slot plan for query block qb.

    Returns list of groups; each group is a list of slots; each slot is
    (subtasks, mask_id) with subtasks a list of (kc, qlo, qhi); kc == -1
    denotes the global-key slot (keys 0:G with standard projections).
    """
    base = 2 * qb
    slots = []
    sub = []
    if base - 2 >= 0:
        sub.append((base - 2, 0, 128))
    if base + 3 < NKC:
        sub.append((base + 3, 128, 256))
    if sub:
        mask = "packed_qb1" if qb == 1 else "packed"
        slots.append((sub, MASK_IDS[mask]))
    for d, mname in ((-1, "m1"), (0, None), (1, None), (2, "m2")):
        kc = base + d
        if 0 <= kc < NKC:
            m = mname
            if kc == 0 and d == 0:
                m = "glob0"          # qb0: zero rows < G (handled globally)
            slots.append(([(kc, 0, 256)], MASK_IDS[m] if m else None))
    slots.append(([(-1, 0, 256)], None))   # global keys
    return [slots[0:3], slots[3:6]]


def _build_masks():
    j = np.arange(128)[:, None]   # key-in-chunk (partition)
    q = np.arange(256)[None, :]   # query-in-block (free)
    packed = np.where(q < 128, j >= q, j <= q - 128)
    packed_qb1 = np.where(q < 128, (j >= q) & (j >= 64), j <= q - 128)
    m1 = (j >= q - 128) & (q < 999)
    m2 = (j <= q) & (q < 999)
    glob0 = (j >= 64) & (q < 999)
    return np.stack([packed, packed_qb1, m1, m2, glob0]).astype(BF16)


def _build_bass():
    nc = bacc.Bacc("TRN2", num_devices=8)

    def inp(name, shape, dtype=dt.bfloat16):
        return nc.declare_dram_parameter(name, list(shape), dtype, isOutput=False)

    xt = inp("xt", [8, 128, 4096])              # [chunk, p, kt*512] host-packed
    wq = inp("wq", [128, 2048])                 # [p, kt*256], pre-scaled 1/8
    wk = inp("wk", [128, 2048])
    wv = inp("wv", [128, 2048])
    wkg = inp("wkg", [128, 2048])
    wvg = inp("wvg", [128, 2048])
    wqg = inp("wqg", [128, 2048])               # pre-scaled by 1/8
    wo = inp("wo", [128, 2048])                 # E-column slice of Wo
    bq = inp("bq", [128, 2], dt.float32)        # pre-scaled by 1/8
    bk = inp("bk", [128, 2], dt.float32)
    bkg = inp("bkg", [128, 2], dt.float32)
    bqg = inp("bqg", [128, 2], dt.float32)      # pre-scaled by 1/8
    bow = inp("bow", [128, 2], dt.float32)      # bo + bv@Wo   (col slice)
    bog = inp("bog", [128, 2], dt.float32)      # bo + bvg@Wo  (col slice)
    masks = inp("masks", [128, 5, 256])         # bf16 0/1 band masks
    outt = nc.declare_dram_parameter("outt", [2, 128, S], dt.float32, isOutput=True)
    dbg = None
    if DEBUG_DUMP:
        dbg = dict(
            dctx=nc.declare_dram_parameter("dctx", [128, 2, S], dt.bfloat16, isOutput=True),
            dv=nc.declare_dram_parameter("dv", [128, 32, 4, 65], dt.bfloat16, isOutput=True),
            dk=nc.declare_dram_parameter("dk", [128, 2, S], dt.bfloat16, isOutput=True),
            dq=nc.declare_dram_parameter("dq", [128, 2, S], dt.bfloat16, isOutput=True))

    with tile.TileContext(nc) as tc:
        _emit(tc, xt, wq, wk, wv, wkg, wvg, wqg, wo, bq, bk, bkg, bqg,
              bow, bog, masks, outt, dbg)
    nc.compile()
    return nc


class Emitter:
    def __init__(self, tc, xt, weights, biases, masks, outt):
        self.nc = tc.nc
        self.tc = tc
        nc = self.nc
        import contextlib
        self.ctx = contextlib.ExitStack()
        ctx = self.ctx

        self.xt, self.masks_in, self.outt = xt, masks, outt

        persist = ctx.enter_context(tc.tile_pool(name="persist", bufs=1))
        self.persist = persist
        self.xchunk = ctx.enter_context(tc.tile_pool(name="xchunk", bufs=2))
        self.ptiles = ctx.enter_context(tc.tile_pool(name="ptiles", bufs=10))
        self.gtiles = ctx.enter_context(tc.tile_pool(name="gtiles", bufs=2))
        self.rbpool = ctx.enter_context(tc.tile_pool(name="rbpool", bufs=2))
        self.otile = ctx.enter_context(tc.tile_pool(name="otile", bufs=2))
        self.dram = ctx.enter_context(tc.tile_pool(name="dram", bufs=2, space="DRAM"))
        self.psum = ctx.enter_context(tc.tile_pool(name="psum", bufs=2, space="PSUM"))

        # ---- weight / bias / mask loads (contiguous per partition) -------
        self.w_sb = {}
        for name, t in weights.items():
            sb = persist.tile([128, 8, 256], dt.bfloat16, name=name + "_sb")
            nc.sync.dma_start(out=sb, in_=t[:].rearrange("p (kt m) -> p kt m", kt=8))
            self.w_sb[name] = sb
        self.b_sb = {}
        for name, t in biases.items():
            sb = persist.tile([128, 2], dt.float32, name=name + "_sb")
            nc.sync.dma_start(out=sb, in_=t[:])
            self.b_sb[name] = sb
        self.mask_sb = persist.tile([128, 5, 256], dt.bfloat16, name="mask_sb")
        nc.sync.dma_start(out=self.mask_sb, in_=masks[:])

        # ---- persistent activation tiles ---------------------------------
        self.qT = persist.tile([128, 2, S], dt.bfloat16, name="qT")
        self.kT = persist.tile([128, 2, S], dt.bfloat16, name="kT")
        self.kgT = persist.tile([128, 2, S], dt.bfloat16, name="kgT")
        self.qgT = persist.tile([128, 2, G], dt.bfloat16, name="qgT")
        self.v_sb = persist.tile([128, 32, 4, 65], dt.bfloat16, name="v")
        self.vg_sb = persist.tile([128, 32, 4, 65], dt.bfloat16, name="vg")
        self.ctxT = persist.tile([128, 2, S], dt.bfloat16, name="ctxT")
        self.ones = persist.tile([128, 64], dt.bfloat16, name="ones")

        nc.vector.memset(self.v_sb[:, :, :, 64:65], 1.0)
        nc.vector.memset(self.vg_sb[:, :, :, 64:65], 1.0)
        nc.vector.memset(self.ones, 1.0)

        self.xc = [None] * 8

    # ---- DMA of an x chunk (sync queue) ----------------------------------
    def load_xc(self, c):
        xc = self.xchunk.tile([128, 8, 512], dt.bfloat16, tag="xc", name="xc")
        self.nc.sync.dma_start(
            out=xc, in_=self.xt[c].rearrange("p (kt s) -> p kt s", kt=8))
        self.xc[c] = xc

    # ---- one transposed projection group: dst[:, hp, cs] -----------------
    def proj_T(self, c, wname, bname, dst, hp):
        nc = self.nc
        cs = slice(c * 512, c * 512 + 512)
        w_sb, b_sb, xc = self.w_sb[wname], self.b_sb[bname], self.xc[c]
        ps = self.psum.tile([128, 512], dt.float32, tag="proj", name="ps_proj")
        for kt in range(8):
            nc.tensor.matmul(
                ps, w_sb[:, kt, hp * 128:hp * 128 + 128],
                xc[:, kt, :], start=(kt == 0), stop=(kt == 7))
        nc.scalar.activation(dst[:, hp, cs], ps, AF.Identity,
                             bias=b_sb[:, hp:hp + 1])

    # ---- one natural-layout v/vg group: dst[:, s32, :, 0:64] -------------
    def proj_V(self, c, wname, dst, sc):
        nc = self.nc
        w_sb, xc = self.w_sb[wname], self.xc[c]
        s32 = c * 4 + sc
        ps = self.psum.tile([128, 512], dt.float32, tag="proj", name="ps_projv")
        psn = ps[:, 0:256]
        for kt in range(8):
            nc.tensor.matmul(
                psn, xc[:, kt, sc * 128:sc * 128 + 128],
                w_sb[:, kt, :], start=(kt == 0), stop=(kt == 7))
        nc.scalar.activation(
            dst[:, s32, :, 0:64],
            psn.rearrange("p (h d) -> p h d", h=4), AF.Copy)

    def proj_qg(self, hp):
        nc = self.nc
        ps = self.psum.tile([128, 512], dt.float32, tag="proj", name="ps_qg")
        psg = ps[:, 0:G]
        for kt in range(8):
            nc.tensor.matmul(
                psg, self.w_sb["wqg"][:, kt, hp * 128:hp * 128 + 128],
                self.xc[0][:, kt, 0:G], start=(kt == 0), stop=(kt == 7))
        nc.scalar.activation(self.qgT[:, hp, :], psg, AF.Identity,
                             bias=self.b_sb["bqg"][:, hp:hp + 1])

    # ---- window attention: QK+exp+masks for one (block, head) ------------
    def window_qk_hi(self, qb, hi):
        nc = self.nc
        q0 = qb * QB
        hp, row = hi // 2, (hi % 2) * 64
        groups = qb_plan(qb)
        pTs = []
        for g in groups:
            n = len(g)
            pss = self.psum.tile([128, 3, QB], dt.float32, tag="scores",
                                 name="pss")
            for si, (subs, mi) in enumerate(g):
                for (kc, qlo, qhi) in subs:
                    if kc == -1:
                        nc.tensor.matmul(
                            pss[0:G, si, qlo:qhi],
                            self.kT[row:row + 64, hp, 0:G],
                            self.qT[row:row + 64, hp, q0 + qlo:q0 + qhi],
                            start=True, stop=True)
                    else:
                        nc.tensor.matmul(
                            pss[:, si, qlo:qhi],
                            self.kT[row:row + 64, hp,
                                    kc * 128:kc * 128 + 128],
                            self.qT[row:row + 64, hp, q0 + qlo:q0 + qhi],
                            start=True, stop=True)
            pT = self.ptiles.tile([128, 3, QB], dt.bfloat16, tag="pT",
                                  name="pT")
            nc.scalar.activation(pT[:, 0:n, :], pss[:, 0:n, :], AF.Exp)
            for si, (subs, mi) in enumerate(g):
                if mi is not None:
                    nc.vector.tensor_mul(pT[:, si, :], pT[:, si, :],
                                         self.mask_sb[:, mi, :])
            pTs.append(pT)
        return (qb, q0, hi, hp, row, groups, pTs)

    # ---- window attention: PV + normalization for all 4 heads ------------
    def window_pv(self, states):
        nc = self.nc
        qb, q0 = states[0][0], states[0][1]
        groups = states[0][5]
        tasks = []
        for gi, g in enumerate(groups):
            for si, (subs, mi) in enumerate(g):
                for (kc, qlo, qhi) in subs:
                    tasks.append((kc, qlo, qhi, gi, si))
        # full-range slots first so PSUM has_written is fully seeded
        tasks.sort(key=lambda t: (t[2] - t[1] != QB))
        lo = G if qb == 0 else 0

        fins = []

        def one_pv(st):
            _, _, hi, hp, row, _, pTs = st
            pv = self.psum.tile([128, QB], dt.float32, tag="pv", name="pv")
            for j, (kc, qlo, qhi, gi, si) in enumerate(tasks):
                pT = pTs[gi]
                if kc == -1:
                    lhs = self.v_sb[0:G, 0, hi, :]
                    rhs = pT[0:G, si, qlo:qhi]
                else:
                    lhs = self.v_sb[:, kc, hi, :]
                    rhs = pT[:, si, qlo:qhi]
                nc.tensor.matmul(pv[0:65, qlo:qhi], lhs, rhs,
                                 start=(j == 0), stop=(j == len(tasks) - 1))
            fins.append((pv, self.ctxT[row:row + 64, hp, q0 + lo:q0 + QB]))

        # lag-1 software pipeline: PV(h0), PV(h1), fin(h0), PV(h2), fin(h1)...
        one_pv(states[0])
        for i in range(1, 4):
            one_pv(states[i])
            self.normalize(*fins[i - 1], lo, QB)
        self.normalize(*fins[3], lo, QB)

    def normalize(self, pv, ctx_out, lo, width):
        """ctx_out = pv[0:64, lo:width] / pv[64, lo:width] (per query).

        The reciprocal of the denominator row is broadcast across the 64
        feature partitions with a 1-row f32r matmul; the unnormalized ctx
        rows are staged to SBUF on the scalar engine (DVE tensor_tensor
        cannot take two PSUM operands), then multiplied on the vector
        engine."""
        nc = self.nc
        tmp = self.rbpool.tile([128, QB], dt.bfloat16, tag="tmp", name="tmp")
        nc.scalar.activation(tmp[0:64, lo:width], pv[0:64, lo:width], AF.Copy)
        rb = self.rbpool.tile([128, QB], dt.float32, tag="rb", name="rb")
        nc.vector.reciprocal(rb[64:65, 0:width], pv[64:65, 0:width])
        rec16 = self.rbpool.tile([128, QB], dt.bfloat16, tag="rec16",
                                 name="rec16")
        nc.scalar.activation(rec16[64:65, 0:width], rb[64:65, 0:width],
                             AF.Copy)
        psB = self.psum.tile([128, 3, QB], dt.float32, tag="scores",
                             name="psB")
        nc.tensor.matmul(
            psB[0:64, 0, 0:width], self.ones[64:65, :],
            rec16[64:65, 0:width], start=True, stop=True)
        nc.vector.tensor_mul(ctx_out, tmp[0:64, lo:width],
                             psB[0:64, 0, lo:width])

    # ---- global-query attention ------------------------------------------
    def glob_qk_t(self, hi, pgT, t):
        """One third of the global-query QK for head hi."""
        nc = self.nc
        hp, row = hi // 2, (hi % 2) * 64
        kcs = list(range(t * 12, min(32, t * 12 + 12)))
        nslot = (len(kcs) + 3) // 4
        pss = self.psum.tile([128, 3, QB], dt.float32, tag="scores",
                             name="pss_g")
        for i, kc in enumerate(kcs):
            si, qo = i // 4, (i % 4) * G
            nc.tensor.matmul(
                pss[:, si, qo:qo + G],
                self.kgT[row:row + 64, hp, kc * 128:kc * 128 + 128],
                self.qgT[row:row + 64, hp, :], start=True, stop=True)
        nc.scalar.activation(
            pgT[:, t * 12:t * 12 + len(kcs), :].rearrange(
                "p k g -> p (k g)").rearrange("p (s q) -> p s q", q=QB),
            pss[:, 0:nslot, :], AF.Exp)

    def glob_pv(self, hi, pgT):
        nc = self.nc
        hp, row = hi // 2, (hi % 2) * 64
        pv = self.psum.tile([128, QB], dt.float32, tag="pv", name="pvg")
        for kc in range(NKC):
            nc.tensor.matmul(pv[0:65, 0:G], self.vg_sb[:, kc, hi, :],
                             pgT[:, kc, :], start=(kc == 0),
                             stop=(kc == NKC - 1))
        self.normalize(pv, self.ctxT[row:row + 64, hp, 0:G], 0, G)

    # ---- gather (+ gctx load) and output projection ----------------------
    def gather(self, cols, tag):
        """AllGather ctxT[:, :, cols] across the 4-core group (gpsimd queue).
        Also issues the gctx load (in-queue after the collective)."""
        nc = self.nc
        width = cols.stop - cols.start
        cc_in = self.dram.tile([256, width], dt.bfloat16, tag="cc_in" + tag,
                               name="cc_in")
        nc.gpsimd.dma_start(
            out=cc_in[:].rearrange("(hp p) s -> p hp s", p=128),
            in_=self.ctxT[:, :, cols])
        cc_out = self.dram.tile([1024, width], dt.bfloat16, tag="cc_out" + tag,
                                name="cc_out")
        nc.gpsimd.collective_compute(
            "AllGather", mybir.AluOpType.bypass,
            replica_groups=[[0, 1, 2, 3], [4, 5, 6, 7]],
            ins=[cc_in[:].opt()], outs=[cc_out[:].opt()])
        gctx = self.gtiles.tile([128, 8, 512], dt.bfloat16, tag="gctx",
                                name="gctx")
        gctx = gctx[:, :, 0:width]
        nc.gpsimd.dma_start(
            out=gctx, in_=cc_out[:].rearrange("(kt p) s -> p kt s", p=128))
        return gctx

    def out_proj(self, gctx, cols, bname):
        nc = self.nc
        width = cols.stop - cols.start
        for mt in range(2):
            pso = self.psum.tile([128, 512], dt.float32, tag="proj",
                                 name="pso")
            pso = pso[:, 0:width]
            for kt in range(8):
                nc.tensor.matmul(
                    pso, self.w_sb["wo"][:, kt, mt * 128:mt * 128 + 128],
                    gctx[:, kt, :], start=(kt == 0), stop=(kt == 7))
            ot = self.otile.tile([128, 512], dt.float32, tag="ot", name="ot")
            ot = ot[:, 0:width]
            nc.scalar.activation(ot, pso, AF.Identity,
                                 bias=self.b_sb[bname][:, mt:mt + 1])
            nc.gpsimd.dma_start(out=self.outt[mt, :, cols], in_=ot)


def _emit(tc, xt, wq, wk, wv, wkg, wvg, wqg, wo, bq, bk, bkg, bqg,
          bow, bog, masks, outt, dbg=None):
    em = Emitter(
        tc, xt,
        dict(wq=wq, wk=wk, wv=wv, wkg=wkg, wvg=wvg, wqg=wqg, wo=wo),
        dict(bq=bq, bk=bk, bkg=bkg, bqg=bqg, bow=bow, bog=bog),
        masks, outt)

    def proj_groups(c):
        """The 14 tensor-queue work units of projection chunk c."""
        gs = []
        for wn, bn, dst in (("wq", "bq", em.qT), ("wk", "bk", em.kT),
                            ("wkg", "bkg", em.kgT)):
            for hp in range(2):
                gs.append(lambda wn=wn, bn=bn, dst=dst, hp=hp:
                          em.proj_T(c, wn, bn, dst, hp))
        for wn, dst in (("wv", em.v_sb), ("wvg", em.vg_sb)):
            for sc in range(4):
                gs.append(lambda wn=wn, dst=dst, sc=sc:
                          em.proj_V(c, wn, dst, sc))
        return gs

    # ---- prologue: chunks 0 and 1, plus qg ------------------------------
    em.load_xc(0)
    em.load_xc(1)
    for g in proj_groups(0):
        g()
    em.proj_qg(0)
    em.proj_qg(1)
    em.load_xc(2)
    for g in proj_groups(1):
        g()

    # ---- steady loop: chunks 2..7 with interleaved windows/gathers ------
    # pending output projections: (gctx, cols, bname)
    pending = []
    chunk_cols = [slice(64, 512)] + [slice(s * 512, s * 512 + 512)
                                     for s in range(1, 8)]
    for c in range(2, 8):
        if c + 1 < 8:
            em.load_xc(c + 1)
        gs = proj_groups(c)
        qa, qbb = 2 * c - 4, 2 * c - 3

        sts = []
        for hi in range(4):              # qb a: QK per head between proj groups
            sts.append(em.window_qk_hi(qa, hi))
            gs.pop(0)()
        em.window_pv(sts)

        sts = []
        for hi in range(4):              # qb b
            sts.append(em.window_qk_hi(qbb, hi))
            gs.pop(0)()
        if pending:
            em.out_proj(*pending.pop(0))
        em.window_pv(sts)

        for g in gs:                     # remaining 6 projection groups
            g()

        s = c - 2                        # output chunk completed this iter
        gctx = em.gather(chunk_cols[s], "a")
        pending.append((gctx, chunk_cols[s], "bow"))

    # ---- post-loop: qb 12..15, global attention, remaining gathers ------
    pgTs = {}
    for i, qb in enumerate((12, 13, 14, 15)):
        pgTs[i] = em.gtiles.tile([128, 32, G], dt.bfloat16, tag="pgT",
                                 name="pgT", bufs=4)
        sts = []
        for hi in range(4):
            sts.append(em.window_qk_hi(qb, hi))
            if hi < 3:
                em.glob_qk_t(i, pgTs[i], hi)
        if pending:
            em.out_proj(*pending.pop(0))
        em.window_pv(sts)
        if qb == 13:
            gctx = em.gather(chunk_cols[6], "a")
            pending.append((gctx, chunk_cols[6], "bow"))

    gctx7 = em.gather(chunk_cols[7], "a")
    for hi in range(4):
        em.glob_pv(hi, pgTs[hi])
    gctx0b = em.gather(slice(0, 64), "b")
    while pending:
        em.out_proj(*pending.pop(0))
    em.out_proj(gctx7, chunk_cols[7], "bow")
    em.out_proj(gctx0b, slice(0, 64), "bog")

    if dbg is not None:
        em.nc.sync.dma_start(out=dbg["dctx"][:], in_=em.ctxT[:])
        em.nc.sync.dma_start(out=dbg["dv"][:], in_=em.v_sb[:])
        em.nc.sync.dma_start(out=dbg["dk"][:], in_=em.kT[:])
        em.nc.sync.dma_start(out=dbg["dq"][:], in_=em.qT[:])

    em.ctx.close()


def _host_inputs(x, Wq, bq, Wk, bk, Wv, Wqg, bqg, Wkg, bkg, Wvg, Wo, bo_w, bo_g):
    """Build the 8 per-core input maps (all pre-packed for contiguous DMA)."""
    masks = np.ascontiguousarray(
        _build_masks().transpose(1, 0, 2))          # [128, 5, 256]

    def b16(a):
        return np.ascontiguousarray(a, dtype=np.float32).astype(BF16)

    def wpack(Wslice):                               # [1024, 256] -> [128, 2048]
        return np.ascontiguousarray(
            b16(Wslice).reshape(8, 128, 256).transpose(1, 0, 2).reshape(128, 2048))

    def bpack(b):                                    # [256] -> [128, 2]
        return np.ascontiguousarray(b.reshape(2, 128).T.astype(np.float32))

    def xpack(xb):                                   # [4096, 1024] -> [8,128,4096]
        xT = b16(xb.T)                               # [1024, 4096]
        return np.ascontiguousarray(
            xT.reshape(8, 128, 8, 512).transpose(2, 1, 0, 3).reshape(8, 128, 4096))

    xP = [xpack(x[b]) for b in range(B)]
    in_maps = []
    for c in range(8):
        b, hg = c // 4, c % 4
        cs = slice(256 * hg, 256 * hg + 256)
        in_maps.append({
            "xt": xP[b],
            "wq": wpack(Wq[:, cs] * 0.125), "wk": wpack(Wk[:, cs]),
            "wv": wpack(Wv[:, cs]), "wkg": wpack(Wkg[:, cs]),
            "wvg": wpack(Wvg[:, cs]), "wqg": wpack(Wqg[:, cs] * 0.125),
            "wo": wpack(Wo[:, cs]),
            "bq": bpack(bq[cs] * 0.125),
            "bk": bpack(bk[cs]),
            "bkg": bpack(bkg[cs]),
            "bqg": bpack(bqg[cs] * 0.125),
            "bow": bpack(bo_w[cs]),
            "bog": bpack(bo_g[cs]),
            "masks": masks.astype(BF16),
        })
    return in_maps


_CACHE = {}


def kernel(hidden_states, key_value_states, Wq, bq, Wk, bk, Wv, bv,
           Wqg, bqg, Wkg, bkg, Wvg, bvg, Wo, bo, num_heads, window,
           num_global, _trace=False):
    x = np.asarray(hidden_states, np.float32)
    args = [np.asarray(a, np.float32) for a in
            (Wq, bq, Wk, bk, Wv, bv, Wqg, bqg, Wkg, bkg, Wvg, bvg, Wo, bo)]
    Wq, bq, Wk, bk, Wv, bv, Wqg, bqg, Wkg, bkg, Wvg, bvg, Wo, bo = args
    bo_w = bo + bv @ Wo
    bo_g = bo + bvg @ Wo

    if "nc" not in _CACHE:
        _CACHE["nc"] = _build_bass()
    nc = _CACHE["nc"]

    in_maps = _host_inputs(x, Wq, bq, Wk, bk, Wv, Wqg, bqg, Wkg, bkg,
                           Wvg, Wo, bo_w, bo_g)
    res = run_bass_kernel_spmd(nc, in_maps, core_ids=list(range(8)),
                               trace=_trace)
    _CACHE["last_result"] = res

    out = np.zeros((B, S, E), np.float32)
    for c in range(8):
        b, hg = c // 4, c % 4
        ot = np.asarray(res.results[c]["outt"], np.float32)  # [2, 128, S]
        out[b, :, 256 * hg:256 * hg + 256] = ot.reshape(256, S).T
    return out


# revision 6
# speedup vs baseline: 1.2305x; 1.0535x over previous
"""Longformer self-attention (BART-style) Trainium2 kernel, v2.

Sharding: 8 cores = 2 batches x 4 head-groups (4 heads each).

v2 redesign vs baseline:
  - Single interleaved emission schedule: window-attention QK blocks (one
    head at a time) are emitted between projection PSUM groups so the
    scalar-engine EXPs hide under projection matmuls and the PE array never
    idles (stays at 2.4GHz).
  - Packed band slots: the two half-masked edge key-chunks (d=-2, d=+3)
    share one 256-wide score slot (query halves); global-key scores ride in
    a normal slot; one EXP per 3-slot group.
  - Softmax normalization via in-SBUF reciprocal + ones-matmul partition
    broadcast + fused multiply into ctxT (no DRAM roundtrip / gpsimd DMA).
  - Host pre-layouts weights/x-chunks so every DMA is one contiguous
    descriptor per partition (sprayed across all 16 DMA engines).
  - Output chunk 0 split into cols [64:512] (gathered early) and [0:64]
    (after global attention) so the AllGather tail is tiny; gather-path DMAs
    ride the gpsimd queue so they never block x-chunk loads.
"""
import sys
import numpy as np

sys.path.insert(0, "/opt/trn_rl_repo")

import ml_dtypes

import concourse.bass as bass
import concourse.bacc as bacc
import concourse.tile as tile
from concourse import mybir
from concourse.bass_utils import run_bass_kernel_spmd

BF16 = ml_dtypes.bfloat16
B, S, E, H, D, W, G = 2, 4096, 1024, 16, 64, 256, 64
QB = 256           # query block for window attention
NKC = S // 128     # 32 key chunks
NQB = S // QB      # 16 query blocks
dt = mybir.dt
AF = mybir.ActivationFunctionType

MASK_IDS = {"packed": 0, "packed_qb1": 1, "m1": 2, "m2": 3, "glob0": 4}

VARIANT = "default"
DEBUG_DUMP = False


def qb_plan(qb):
    """Slot plan for query block qb.

    Returns list of groups; each group is a list of slots; each slot is
    (subtasks, mask_id) with subtasks a list of (kc, qlo, qhi); kc == -1
    denotes the global-key slot (keys 0:G with standard projections).
    """
    base = 2 * qb
    slots = []
    sub = []
    if base - 2 >= 0:
        sub.append((base - 2, 0, 128))
    if base + 3 < NKC:
        sub.append((base + 3, 128, 256))
    if sub:
        mask = "packed_qb1" if qb == 1 else "packed"
        slots.append((sub, MASK_IDS[mask]))
    for d, mname in ((-1, "m1"), (0, None), (1, None), (2, "m2")):
        kc = base + d
        if 0 <= kc < NKC:
            m = mname
            if kc == 0 and d == 0:
                m = "glob0"          # qb0: zero rows < G (handled globally)
            slots.append(([(kc, 0, 256)], MASK_IDS[m] if m else None))
    slots.append(([(-1, 0, 256)], None))   # global keys
    return [slots[0:3], slots[3:6]]


def _build_masks():
    j = np.arange(128)[:, None]   # key-in-chunk (partition)
    q = np.arange(256)[None, :]   # query-in-block (free)
    packed = np.where(q < 128, j >= q, j <= q - 128)
    packed_qb1 = np.where(q < 128, (j >= q) & (j >= 64), j <= q - 128)
    m1 = (j >= q - 128) & (q < 999)
    m2 = (j <= q) & (q < 999)
    glob0 = (j >= 64) & (q < 999)
    return np.stack([packed, packed_qb1, m1, m2, glob0]).astype(BF16)


def _build_bass():
    nc = bacc.Bacc("TRN2", num_devices=8)

    def inp(name, shape, dtype=dt.bfloat16):
        return nc.declare_dram_parameter(name, list(shape), dtype, isOutput=False)

    xt = inp("xt", [8, 128, 4096])              # [chunk, p, kt*512] host-packed
    wq = inp("wq", [128, 2048])                 # [p, kt*256], pre-scaled 1/8
    wk = inp("wk", [128, 2048])
    wv = inp("wv", [128, 2048])
    wkg = inp("wkg", [128, 2048])
    wvg = inp("wvg", [128, 2048])
    wqg = inp("wqg", [128, 2048])               # pre-scaled by 1/8
    wo = inp("wo", [128, 2048])                 # E-column slice of Wo
    bq = inp("bq", [128, 2], dt.float32)        # pre-scaled by 1/8
    bk = inp("bk", [128, 2], dt.float32)
    bkg = inp("bkg", [128, 2], dt.float32)
    bqg = inp("bqg", [128, 2], dt.float32)      # pre-scaled by 1/8
    bow = inp("bow", [128, 2], dt.float32)      # bo + bv@Wo   (col slice)
    bog = inp("bog", [128, 2], dt.float32)      # bo + bvg@Wo  (col slice)
    masks = inp("masks", [128, 5, 256])         # bf16 0/1 band masks
    outt = nc.declare_dram_parameter("outt", [2, 128, S], dt.float32, isOutput=True)
    dbg = None
    if DEBUG_DUMP:
        dbg = dict(
            dctx=nc.declare_dram_parameter("dctx", [128, 2, S], dt.bfloat16, isOutput=True),
            dv=nc.declare_dram_parameter("dv", [128, 32, 4, 65], dt.bfloat16, isOutput=True),
            dk=nc.declare_dram_parameter("dk", [128, 2, S], dt.bfloat16, isOutput=True),
            dq=nc.declare_dram_parameter("dq", [128, 2, S], dt.bfloat16, isOutput=True))

    with tile.TileContext(nc) as tc:
        _emit(tc, xt, wq, wk, wv, wkg, wvg, wqg, wo, bq, bk, bkg, bqg,
              bow, bog, masks, outt, dbg)
    nc.compile()
    return nc


class Emitter:
    def __init__(self, tc, xt, weights, biases, masks, outt):
        self.nc = tc.nc
        self.tc = tc
        nc = self.nc
        import contextlib
        self.ctx = contextlib.ExitStack()
        ctx = self.ctx

        self.xt, self.masks_in, self.outt = xt, masks, outt

        persist = ctx.enter_context(tc.tile_pool(name="persist", bufs=1))
        self.persist = persist
        self.xchunk = ctx.enter_context(tc.tile_pool(name="xchunk", bufs=2))
        self.ptiles = ctx.enter_context(tc.tile_pool(name="ptiles", bufs=9))
        self.gtiles = ctx.enter_context(tc.tile_pool(name="gtiles", bufs=2))
        self.rbpool = ctx.enter_context(tc.tile_pool(name="rbpool", bufs=2))
        self.otile = ctx.enter_context(tc.tile_pool(name="otile", bufs=2))
        self.dram = ctx.enter_context(tc.tile_pool(name="dram", bufs=2, space="DRAM"))
        self.psum = ctx.enter_context(tc.tile_pool(name="psum", bufs=2, space="PSUM"))

        # ---- weight / bias / mask loads (contiguous per partition) -------
        self.w_sb = {}
        for name, t in weights.items():
            sb = persist.tile([128, 8, 256], dt.bfloat16, name=name + "_sb")
            nc.sync.dma_start(out=sb, in_=t[:].rearrange("p (kt m) -> p kt m", kt=8))
            self.w_sb[name] = sb
        self.b_sb = {}
        for name, t in biases.items():
            sb = persist.tile([128, 2], dt.float32, name=name + "_sb")
            nc.sync.dma_start(out=sb, in_=t[:])
            self.b_sb[name] = sb
        self.mask_sb = persist.tile([128, 5, 256], dt.bfloat16, name="mask_sb")
        nc.sync.dma_start(out=self.mask_sb, in_=masks[:])

        # ---- persistent activation tiles ---------------------------------
        self.qT = persist.tile([128, 2, S], dt.bfloat16, name="qT")
        self.kT = persist.tile([128, 2, S], dt.bfloat16, name="kT")
        self.kgT = persist.tile([128, 2, S], dt.bfloat16, name="kgT")
        self.qgT = persist.tile([128, 2, G], dt.bfloat16, name="qgT")
        self.v_sb = persist.tile([128, 32, 4, 65], dt.bfloat16, name="v")
        self.vg_sb = persist.tile([128, 32, 4, 65], dt.bfloat16, name="vg")
        self.ctxT = persist.tile([128, 2, S], dt.bfloat16, name="ctxT")
        self.ones = persist.tile([128, 64], dt.bfloat16, name="ones")

        nc.vector.memset(self.v_sb[:, :, :, 64:65], 1.0)
        nc.vector.memset(self.vg_sb[:, :, :, 64:65], 1.0)
        nc.vector.memset(self.ones, 1.0)

        self.xc = [None] * 8

    # ---- DMA of an x chunk (sync queue) ----------------------------------
    def load_xc(self, c):
        xc = self.xchunk.tile([128, 8, 512], dt.bfloat16, tag="xc", name="xc")
        self.nc.sync.dma_start(
            out=xc, in_=self.xt[c].rearrange("p (kt s) -> p kt s", kt=8))
        self.xc[c] = xc

    # ---- one transposed projection group: dst[:, hp, cs] -----------------
    def proj_T(self, c, wname, bname, dst, hp):
        nc = self.nc
        cs = slice(c * 512, c * 512 + 512)
        w_sb, b_sb, xc = self.w_sb[wname], self.b_sb[bname], self.xc[c]
        ps = self.psum.tile([128, 512], dt.float32, tag="proj", name="ps_proj")
        for kt in range(8):
            nc.tensor.matmul(
                ps, w_sb[:, kt, hp * 128:hp * 128 + 128],
                xc[:, kt, :], start=(kt == 0), stop=(kt == 7))
        nc.scalar.activation(dst[:, hp, cs], ps, AF.Identity,
                             bias=b_sb[:, hp:hp + 1])

    # ---- one natural-layout v/vg group: dst[:, s32, :, 0:64] -------------
    def proj_V(self, c, wname, dst, sc):
        nc = self.nc
        w_sb, xc = self.w_sb[wname], self.xc[c]
        s32 = c * 4 + sc
        ps = self.psum.tile([128, 512], dt.float32, tag="proj", name="ps_projv")
        psn = ps[:, 0:256]
        for kt in range(8):
            nc.tensor.matmul(
                psn, xc[:, kt, sc * 128:sc * 128 + 128],
                w_sb[:, kt, :], start=(kt == 0), stop=(kt == 7))
        nc.scalar.activation(
            dst[:, s32, :, 0:64],
            psn.rearrange("p (h d) -> p h d", h=4), AF.Copy)

    def proj_qg(self, hp):
        nc = self.nc
        ps = self.psum.tile([128, 512], dt.float32, tag="proj", name="ps_qg")
        psg = ps[:, 0:G]
        for kt in range(8):
            nc.tensor.matmul(
                psg, self.w_sb["wqg"][:, kt, hp * 128:hp * 128 + 128],
                self.xc[0][:, kt, 0:G], start=(kt == 0), stop=(kt == 7))
        nc.scalar.activation(self.qgT[:, hp, :], psg, AF.Identity,
                             bias=self.b_sb["bqg"][:, hp:hp + 1])

    # ---- window attention: QK+exp+masks for one (block, head) ------------
    def window_qk_hi(self, qb, hi):
        nc = self.nc
        q0 = qb * QB
        hp, row = hi // 2, (hi % 2) * 64
        groups = qb_plan(qb)
        pTs = []
        for g in groups:
            n = len(g)
            pss = self.psum.tile([128, 3, QB], dt.float32, tag="scores",
                                 name="pss")
            for si, (subs, mi) in enumerate(g):
                for (kc, qlo, qhi) in subs:
                    if kc == -1:
                        nc.tensor.matmul(
                            pss[0:G, si, qlo:qhi],
                            self.kT[row:row + 64, hp, 0:G],
                            self.qT[row:row + 64, hp, q0 + qlo:q0 + qhi],
                            start=True, stop=True)
                    else:
                        nc.tensor.matmul(
                            pss[:, si, qlo:qhi],
                            self.kT[row:row + 64, hp,
                                    kc * 128:kc * 128 + 128],
                            self.qT[row:row + 64, hp, q0 + qlo:q0 + qhi],
                            start=True, stop=True)
            pT = self.ptiles.tile([128, 3, QB], dt.bfloat16, tag="pT",
                                  name="pT")
            nc.scalar.activation(pT[:, 0:n, :], pss[:, 0:n, :], AF.Exp)
            for si, (subs, mi) in enumerate(g):
                if mi is not None:
                    nc.vector.tensor_mul(pT[:, si, :], pT[:, si, :],
                                         self.mask_sb[:, mi, :])
            pTs.append(pT)
        return (qb, q0, hi, hp, row, groups, pTs)

    # ---- window attention: PV + normalization for all 4 heads ------------
    def window_pv(self, states):
        nc = self.nc
        qb, q0 = states[0][0], states[0][1]
        groups = states[0][5]
        tasks = []
        for gi, g in enumerate(groups):
            for si, (subs, mi) in enumerate(g):
                for (kc, qlo, qhi) in subs:
                    tasks.append((kc, qlo, qhi, gi, si))
        # full-range slots first so PSUM has_written is fully seeded
        tasks.sort(key=lambda t: (t[2] - t[1] != QB))
        lo = G if qb == 0 else 0

        fins = []

        def one_pv(st):
            _, _, hi, hp, row, _, pTs = st
            pv = self.psum.tile([128, QB], dt.float32, tag="pv", name="pv")
            for j, (kc, qlo, qhi, gi, si) in enumerate(tasks):
                pT = pTs[gi]
                if kc == -1:
                    lhs = self.v_sb[0:G, 0, hi, :]
                    rhs = pT[0:G, si, qlo:qhi]
                else:
                    lhs = self.v_sb[:, kc, hi, :]
                    rhs = pT[:, si, qlo:qhi]
                nc.tensor.matmul(pv[0:65, qlo:qhi], lhs, rhs,
                                 start=(j == 0), stop=(j == len(tasks) - 1))
            self.stage_fin(fins, pv,
                           self.ctxT[row:row + 64, hp, q0 + lo:q0 + QB],
                           lo, QB)

        for i in range(4):
            one_pv(states[i])
        self.normalize_batch(fins, lo, QB)

    def stage_fin(self, fins, pv, ctx_out, lo, width):
        """Drain pv (PSUM) to SBUF right away: unnormalized ctx rows to a
        bf16 tmp tile (scalar) and the denominator row to partition 32*i of
        a shared staging tile (scalar copy + tiny sbuf-to-sbuf DMA).  The
        PSUM buffer is then free for the next head."""
        nc = self.nc
        i = len(fins)
        if i == 0:
            self._d64 = self.rbpool.tile([128, 4, QB], dt.bfloat16,
                                         tag="d64", name="d64")
            self._dens = self.rbpool.tile([128, 2, QB], dt.bfloat16,
                                          tag="dens", name="dens")
        tmp = self.rbpool.tile([128, QB], dt.bfloat16, tag="tmp",
                               name="tmp", bufs=4)
        nc.scalar.activation(tmp[0:64, lo:width], pv[0:64, lo:width],
                             AF.Copy)
        nc.scalar.activation(self._d64[64:65, i, 0:width],
                             pv[64:65, 0:width], AF.Copy)
        p, sl = 32 * (i % 3), i // 3
        nc.scalar.dma_start(out=self._dens[p:p + 1, sl, 0:width],
                            in_=self._d64[64:65, i, 0:width])
        fins.append((tmp, ctx_out))

    def normalize_batch(self, fins, lo, width):
        """One (slow, 8-cycle/element) DVE reciprocal serves all four
        heads' denominators; each head then gets a bf16 cast + ones-matmul
        partition broadcast + one vector multiply."""
        nc = self.nc
        recs = self.rbpool.tile([128, 2, QB], dt.float32, tag="recs",
                                name="recs")
        nc.vector.reciprocal(recs[:, :, 0:width], self._dens[:, :, 0:width])
        rec16 = self.rbpool.tile([128, 2, QB], dt.bfloat16, tag="rec16",
                                 name="rec16")
        for hi, (tmp, ctx_out) in enumerate(fins):
            p, sl = 32 * (hi % 3), hi // 3
            r = slice(p, p + 1)
            nc.scalar.activation(rec16[r, sl, 0:width],
                                 recs[r, sl, 0:width], AF.Copy)
            psB = self.psum.tile([128, 3, QB], dt.float32, tag="scores",
                                 name="psB")
            nc.tensor.matmul(
                psB[0:64, 0, 0:width], self.ones[r, :],
                rec16[r, sl, 0:width], start=True, stop=True)
            nc.vector.tensor_mul(ctx_out, tmp[0:64, lo:width],
                                 psB[0:64, 0, lo:width])

    # ---- global-query attention ------------------------------------------
    def glob_qk_t(self, hi, pgT, t):
        """One third of the global-query QK for head hi."""
        nc = self.nc
        hp, row = hi // 2, (hi % 2) * 64
        kcs = list(range(t * 12, min(32, t * 12 + 12)))
        nslot = (len(kcs) + 3) // 4
        pss = self.psum.tile([128, 3, QB], dt.float32, tag="scores",
                             name="pss_g")
        for i, kc in enumerate(kcs):
            si, qo = i // 4, (i % 4) * G
            nc.tensor.matmul(
                pss[:, si, qo:qo + G],
                self.kgT[row:row + 64, hp, kc * 128:kc * 128 + 128],
                self.qgT[row:row + 64, hp, :], start=True, stop=True)
        nc.scalar.activation(
            pgT[:, t * 12:t * 12 + len(kcs), :].rearrange(
                "p k g -> p (k g)").rearrange("p (s q) -> p s q", q=QB),
            pss[:, 0:nslot, :], AF.Exp)

    def glob_pv(self, hi, pgT, fins):
        nc = self.nc
        hp, row = hi // 2, (hi % 2) * 64
        pv = self.psum.tile([128, QB], dt.float32, tag="pv", name="pvg")
        for kc in range(NKC):
            nc.tensor.matmul(pv[0:65, 0:G], self.vg_sb[:, kc, hi, :],
                             pgT[:, kc, :], start=(kc == 0),
                             stop=(kc == NKC - 1))
        self.stage_fin(fins, pv, self.ctxT[row:row + 64, hp, 0:G], 0, G)

    # ---- gather (+ gctx load) and output projection ----------------------
    def gather(self, cols, tag):
        """AllGather ctxT[:, :, cols] across the 4-core group (gpsimd queue).
        Also issues the gctx load (in-queue after the collective)."""
        nc = self.nc
        width = cols.stop - cols.start
        cc_in = self.dram.tile([256, width], dt.bfloat16, tag="cc_in" + tag,
                               name="cc_in")
        nc.gpsimd.dma_start(
            out=cc_in[:].rearrange("(hp p) s -> p hp s", p=128),
            in_=self.ctxT[:, :, cols])
        cc_out = self.dram.tile([1024, width], dt.bfloat16, tag="cc_out" + tag,
                                name="cc_out")
        nc.gpsimd.collective_compute(
            "AllGather", mybir.AluOpType.bypass,
            replica_groups=[[0, 1, 2, 3], [4, 5, 6, 7]],
            ins=[cc_in[:].opt()], outs=[cc_out[:].opt()])
        gctx = self.gtiles.tile([128, 8, 512], dt.bfloat16, tag="gctx",
                                name="gctx")
        gctx = gctx[:, :, 0:width]
        nc.gpsimd.dma_start(
            out=gctx, in_=cc_out[:].rearrange("(kt p) s -> p kt s", p=128))
        return gctx

    def out_proj(self, gctx, cols, bname):
        nc = self.nc
        width = cols.stop - cols.start
        for mt in range(2):
            pso = self.psum.tile([128, 512], dt.float32, tag="proj",
                                 name="pso")
            pso = pso[:, 0:width]
            for kt in range(8):
                nc.tensor.matmul(
                    pso, self.w_sb["wo"][:, kt, mt * 128:mt * 128 + 128],
                    gctx[:, kt, :], start=(kt == 0), stop=(kt == 7))
            ot = self.otile.tile([128, 512], dt.float32, tag="ot", name="ot")
            ot = ot[:, 0:width]
            nc.scalar.activation(ot, pso, AF.Identity,
                                 bias=self.b_sb[bname][:, mt:mt + 1])
            nc.gpsimd.dma_start(out=self.outt[mt, :, cols], in_=ot)


def _emit(tc, xt, wq, wk, wv, wkg, wvg, wqg, wo, bq, bk, bkg, bqg,
          bow, bog, masks, outt, dbg=None):
    em = Emitter(
        tc, xt,
        dict(wq=wq, wk=wk, wv=wv, wkg=wkg, wvg=wvg, wqg=wqg, wo=wo),
        dict(bq=bq, bk=bk, bkg=bkg, bqg=bqg, bow=bow, bog=bog),
        masks, outt)

    def proj_groups(c):
        """The 14 tensor-queue work units of projection chunk c."""
        gs = []
        for wn, bn, dst in (("wq", "bq", em.qT), ("wk", "bk", em.kT),
                            ("wkg", "bkg", em.kgT)):
            for hp in range(2):
                gs.append(lambda wn=wn, bn=bn, dst=dst, hp=hp:
                          em.proj_T(c, wn, bn, dst, hp))
        for wn, dst in (("wv", em.v_sb), ("wvg", em.vg_sb)):
            for sc in range(4):
                gs.append(lambda wn=wn, dst=dst, sc=sc:
                          em.proj_V(c, wn, dst, sc))
        return gs

    # ---- prologue: chunks 0 and 1, plus qg ------------------------------
    em.load_xc(0)
    em.load_xc(1)
    for g in proj_groups(0):
        g()
    em.proj_qg(0)
    em.proj_qg(1)
    em.load_xc(2)
    for g in proj_groups(1):
        g()

    # ---- steady loop: chunks 2..7 with interleaved windows/gathers ------
    # pending output projections: (gctx, cols, bname)
    pending = []
    chunk_cols = [slice(64, 512)] + [slice(s * 512, s * 512 + 512)
                                     for s in range(1, 8)]
    for c in range(2, 8):
        if c + 1 < 8:
            em.load_xc(c + 1)
        gs = proj_groups(c)
        qa, qbb = 2 * c - 4, 2 * c - 3

        sts = []
        for hi in range(4):              # qb a: QK per head between proj groups
            sts.append(em.window_qk_hi(qa, hi))
            gs.pop(0)()
        em.window_pv(sts)

        sts = []
        for hi in range(4):              # qb b
            sts.append(em.window_qk_hi(qbb, hi))
            gs.pop(0)()
        if pending:
            em.out_proj(*pending.pop(0))
        em.window_pv(sts)

        for g in gs:                     # remaining 6 projection groups
            g()

        s = c - 2                        # output chunk completed this iter
        gctx = em.gather(chunk_cols[s], "a")
        pending.append((gctx, chunk_cols[s], "bow"))

    # ---- post-loop: qb 12..15, global attention, remaining gathers ------
    pgTs = {}
    for i, qb in enumerate((12, 13, 14, 15)):
        pgTs[i] = em.gtiles.tile([128, 32, G], dt.bfloat16, tag="pgT",
                                 name="pgT", bufs=4)
        sts = []
        for hi in range(4):
            sts.append(em.window_qk_hi(qb, hi))
            if hi < 3:
                em.glob_qk_t(i, pgTs[i], hi)
        if pending:
            em.out_proj(*pending.pop(0))
        em.window_pv(sts)
        if qb == 13:
            gctx = em.gather(chunk_cols[6], "a")
            pending.append((gctx, chunk_cols[6], "bow"))

    gctx7 = em.gather(chunk_cols[7], "a")
    gfins = []
    for hi in range(4):
        em.glob_pv(hi, pgTs[hi], gfins)
    em.normalize_batch(gfins, 0, G)
    gctx0b = em.gather(slice(0, 64), "b")
    while pending:
        em.out_proj(*pending.pop(0))
    em.out_proj(gctx7, chunk_cols[7], "bow")
    em.out_proj(gctx0b, slice(0, 64), "bog")

    if dbg is not None:
        em.nc.sync.dma_start(out=dbg["dctx"][:], in_=em.ctxT[:])
        em.nc.sync.dma_start(out=dbg["dv"][:], in_=em.v_sb[:])
        em.nc.sync.dma_start(out=dbg["dk"][:], in_=em.kT[:])
        em.nc.sync.dma_start(out=dbg["dq"][:], in_=em.qT[:])

    em.ctx.close()


def _host_inputs(x, Wq, bq, Wk, bk, Wv, Wqg, bqg, Wkg, bkg, Wvg, Wo, bo_w, bo_g):
    """Build the 8 per-core input maps (all pre-packed for contiguous DMA)."""
    masks = np.ascontiguousarray(
        _build_masks().transpose(1, 0, 2))          # [128, 5, 256]

    def b16(a):
        return np.ascontiguousarray(a, dtype=np.float32).astype(BF16)

    def wpack(Wslice):                               # [1024, 256] -> [128, 2048]
        return np.ascontiguousarray(
            b16(Wslice).reshape(8, 128, 256).transpose(1, 0, 2).reshape(128, 2048))

    def bpack(b):                                    # [256] -> [128, 2]
        return np.ascontiguousarray(b.reshape(2, 128).T.astype(np.float32))

    def xpack(xb):                                   # [4096, 1024] -> [8,128,4096]
        xT = b16(xb.T)                               # [1024, 4096]
        return np.ascontiguousarray(
            xT.reshape(8, 128, 8, 512).transpose(2, 1, 0, 3).reshape(8, 128, 4096))

    xP = [xpack(x[b]) for b in range(B)]
    in_maps = []
    for c in range(8):
        b, hg = c // 4, c % 4
        cs = slice(256 * hg, 256 * hg + 256)
        in_maps.append({
            "xt": xP[b],
            "wq": wpack(Wq[:, cs] * 0.125), "wk": wpack(Wk[:, cs]),
            "wv": wpack(Wv[:, cs]), "wkg": wpack(Wkg[:, cs]),
            "wvg": wpack(Wvg[:, cs]), "wqg": wpack(Wqg[:, cs] * 0.125),
            "wo": wpack(Wo[:, cs]),
            "bq": bpack(bq[cs] * 0.125),
            "bk": bpack(bk[cs]),
            "bkg": bpack(bkg[cs]),
            "bqg": bpack(bqg[cs] * 0.125),
            "bow": bpack(bo_w[cs]),
            "bog": bpack(bo_g[cs]),
            "masks": masks.astype(BF16),
        })
    return in_maps


_CACHE = {}


def kernel(hidden_states, key_value_states, Wq, bq, Wk, bk, Wv, bv,
           Wqg, bqg, Wkg, bkg, Wvg, bvg, Wo, bo, num_heads, window,
           num_global, _trace=False):
    x = np.asarray(hidden_states, np.float32)
    args = [np.asarray(a, np.float32) for a in
            (Wq, bq, Wk, bk, Wv, bv, Wqg, bqg, Wkg, bkg, Wvg, bvg, Wo, bo)]
    Wq, bq, Wk, bk, Wv, bv, Wqg, bqg, Wkg, bkg, Wvg, bvg, Wo, bo = args
    bo_w = bo + bv @ Wo
    bo_g = bo + bvg @ Wo

    if "nc" not in _CACHE:
        _CACHE["nc"] = _build_bass()
    nc = _CACHE["nc"]

    in_maps = _host_inputs(x, Wq, bq, Wk, bk, Wv, Wqg, bqg, Wkg, bkg,
                           Wvg, Wo, bo_w, bo_g)
    res = run_bass_kernel_spmd(nc, in_maps, core_ids=list(range(8)),
                               trace=_trace)
    _CACHE["last_result"] = res

    out = np.zeros((B, S, E), np.float32)
    for c in range(8):
        b, hg = c // 4, c % 4
        ot = np.asarray(res.results[c]["outt"], np.float32)  # [2, 128, S]
        out[b, :, 256 * hg:256 * hg + 256] = ot.reshape(256, S).T
    return out
